# revision 12
# baseline (speedup 1.0000x reference)
"""Pairwise-affinity kernel for Trainium2: adj[i,j] = exp(-||x_i - x_j||_2 / T).

Row-parallel over 8 NeuronCores: core c computes the [N/8, N] slab for rows
c*N/8 .. (c+1)*N/8.

Per-core math, all in one PSUM accumulation group per output tile:
    psum = S^2*(sq_i + sq_j) - 2 * xs_i . xs_j          (scaled units)
  - xs = fp8_e4m3(S*x), S = 8192 (power of two; lifts values well out of the
    fp8 subnormal range, max |2*S*x| ~ 80 << 240). Matmuls run in fp8
    DoubleRow perf mode: both operands carry 2 k-planes as [128, 2, free]
    APs, K=256 per instruction, ~1.44x measured over bf16.
  - The -2 is folded into the lhsT operand (exact: power-of-two scale).
  - sq = rowsum(xs^2) in fp32, injected via a K=6 bf16 rank-update matmul
    using a 3-term bf16 hi/mid/lo split of sq (keeps the diagonal of d2 at
    fp32-roundoff scale so exp(-100*sqrt(d2)) stays ~1 on the diagonal, like
    the fp32 reference).
Epilogue per tile: DVE relu-clamp, then ACT Sqrt and ACT Exp(-100/S *).
Sqrt and Exp live in different ACT table sets (~2.7us reload per switch), so
the two passes are batched per column-group: all 8 m-tiles' Sqrts run with
the sqrt set resident, then all 8 Exps with the exp set -- 2 reloads per
group instead of 2 per tile. (HW-measured: the Sqrt table is good to ~7e-6
rel despite its loose ULP budget, and Sqrt(0) = 0 exactly.)
"""

import numpy as np
import ml_dtypes

N = 8192
D = 1024
NCORES = 8
NLOC = N // NCORES  # 1024
TEMP = 0.01
S = 8192.0          # fp8 input scale (power of two)

P = 128          # partitions
NW = 512         # matmul moving-operand width (one PSUM bank of fp32)
PSW = 2 * NW     # psum tile width (2 banks) = DVE op width
KT2 = D // (2 * P)   # 4 DoubleRow k-steps (K=256 each)
MT = NLOC // P       # 8 m-tiles
GPM = N // (2 * PSW)  # 4 column groups per m-tile, 2048 cols each

_cache = {}


def _build():
    from contextlib import ExitStack

    import concourse.bacc as bacc
    import concourse.tile as tile
    from concourse import mybir

    dt = mybir.dt
    nc = bacc.Bacc("TRN2", target_bir_lowering=False, debug=False,
                   num_devices=NCORES)

    xt8 = nc.dram_tensor("xt8", [KT2, P, 2, N], dt.float8e4,
                         kind="ExternalInput")
    lt8 = nc.dram_tensor("lt8", [KT2, P, 2, NLOC], dt.float8e4,
                         kind="ExternalInput")
    il = nc.dram_tensor("il", [6, NLOC], dt.bfloat16, kind="ExternalInput")
    ir = nc.dram_tensor("ir", [6, N], dt.bfloat16, kind="ExternalInput")
    out = nc.dram_tensor("out", [NLOC, N], dt.float32, kind="ExternalOutput")

    GW = 2 * PSW  # columns per (m, g) group: 2048
    DR = mybir.MatmulPerfMode.DoubleRow

    with tile.TileContext(nc) as tc, ExitStack() as ctx:
        xt_pool = ctx.enter_context(tc.tile_pool(name="xt_pool", bufs=1))
        lt_pool = ctx.enter_context(tc.tile_pool(name="lt_pool", bufs=1))
        ini_pool = ctx.enter_context(tc.tile_pool(name="ini_pool", bufs=1))
        psum_pool = ctx.enter_context(
            tc.tile_pool(name="psum_pool", bufs=4, space="PSUM"))
        d2_pool = ctx.enter_context(tc.tile_pool(name="d2_pool", bufs=10))

        # Small operands first so the K=6 seed matmuls can issue early.
        ilt = ini_pool.tile([6, NLOC], dt.bfloat16, name="ilt", tag="ilt")
        nc.sync.dma_start(ilt[:], il[:])
        irt = ini_pool.tile([6, N], dt.bfloat16, name="irt", tag="irt")
        nc.sync.dma_start(irt[:], ir[:])
        lts = []
        for k in range(KT2):
            t = lt_pool.tile([P, 2, NLOC], dt.float8e4,
                             name=f"lts{k}", tag=f"lts{k}")
            nc.sync.dma_start(t[:], lt8[k])
            lts.append(t)

        # fp8 xsT loads chunked by column group: group g's 2MB unlocks all 8
        # m-tiles of PE work for those columns within ~10us.
        xts = []
        for k in range(KT2):
            t = xt_pool.tile([P, 2, N], dt.float8e4,
                             name=f"xts{k}", tag=f"xts{k}")
            xts.append(t)
        for g in range(GPM):
            base = g * GW
            for k in range(KT2):
                nc.sync.dma_start(xts[k][:, :, base:base + GW],
                                  xt8[k, :, :, base:base + GW])

        for g in range(GPM):
            base = g * GW
            d2ts = []
            for m in range(MT):
                ilm = ilt[:, m * P:(m + 1) * P]
                ps = []
                for t_ in range(2):
                    pst = psum_pool.tile([P, PSW], dt.float32,
                                         name=f"ps{m}_{g}_{t_}", tag="ps")
                    ps.append(pst)
                # K=6 bf16 rank-update seeds psum with S^2*(sq_i + sq_j),
                # then 4 fp8 DoubleRow k-steps accumulate -2*xs.xs.
                for t_ in range(2):
                    for h in range(2):
                        cw = base + t_ * PSW + h * NW
                        nc.tensor.matmul(ps[t_][:, h * NW:(h + 1) * NW],
                                         ilm, irt[:, cw:cw + NW],
                                         start=True, stop=False)
                for k in range(KT2):
                    lk = lts[k][:, :, m * P:(m + 1) * P]
                    last = k == KT2 - 1
                    for t_ in range(2):
                        for h in range(2):
                            cw = base + t_ * PSW + h * NW
                            nc.tensor.matmul(ps[t_][:, h * NW:(h + 1) * NW],
                                             lk, xts[k][:, :, cw:cw + NW],
                                             start=False, stop=last,
                                             perf_mode=DR)
                # Relu-clamp (frees the psum banks) + Sqrt, in place.
                d2t = d2_pool.tile([P, GW], dt.float32,
                                   name=f"d2_{m}_{g}", tag="d2")
                for t_ in range(2):
                    nc.vector.tensor_scalar(
                        d2t[:, t_ * PSW:(t_ + 1) * PSW], ps[t_][:],
                        0.0, None, mybir.AluOpType.max)
                nc.scalar.activation(d2t[:], d2t[:],
                                     mybir.ActivationFunctionType.Sqrt)
                d2ts.append(d2t)
            # Second ACT pass for the whole column group: one table switch
            # to the exp set (and one back to sqrt at the next group).
            for m in range(MT):
                d2t = d2ts[m]
                nc.scalar.activation(d2t[:], d2t[:],
                                     mybir.ActivationFunctionType.Exp,
                                     scale=-1.0 / (TEMP * S))
                nc.sync.dma_start(out[m * P:(m + 1) * P, base:base + GW],
                                  d2t[:])

    nc.compile()
    return nc


def _get_nc():
    if "nc" not in _cache:
        _cache["nc"] = _build()
    return _cache["nc"]


def _pack_dr(a8):
    """[Dk, W] fp8 (k-major) -> [KT2, P, 2, W] DoubleRow-packed, contiguous.

    k = k2*256 + i*128 + p maps to [k2, p, i, :]."""
    w = a8.shape[1]
    return np.ascontiguousarray(
        a8.reshape(KT2, 2, P, w).transpose(0, 2, 1, 3))


def _prep_inputs(X: np.ndarray):
    bf16 = ml_dtypes.bfloat16
    f8 = ml_dtypes.float8_e4m3
    Xs = (X * S).astype(f8)                   # xs [N, D]
    Xs32 = Xs.astype(np.float32)
    sq = np.einsum("ij,ij->i", Xs32, Xs32, dtype=np.float32)  # S^2 * sq-ish

    h = sq.astype(bf16)
    r = sq - h.astype(np.float32)
    md = r.astype(bf16)
    l = (r - md.astype(np.float32)).astype(bf16)
    ones = np.ones(N, dtype=bf16)

    xt8_full = _pack_dr(np.ascontiguousarray(Xs.T))     # [KT2, P, 2, N]
    ir_full = np.ascontiguousarray(
        np.stack([ones, ones, ones, h, md, l]))          # [6, N]

    in_maps = []
    for c in range(NCORES):
        rows = slice(c * NLOC, (c + 1) * NLOC)
        lt8c = _pack_dr(np.ascontiguousarray(
            (Xs32[rows] * -2.0).astype(f8).T))           # [KT2, P, 2, NLOC]
        ilc = np.ascontiguousarray(
            np.stack([h[rows], md[rows], l[rows],
                      ones[rows], ones[rows], ones[rows]]))  # [6, NLOC]
        in_maps.append({"xt8": xt8_full, "lt8": lt8c, "il": ilc,
                       "ir": ir_full})
    return in_maps


def _run(X: np.ndarray, trace: bool = False):
    from concourse.bass_utils import run_bass_kernel_spmd

    nc = _get_nc()
    in_maps = _prep_inputs(X)
    try:
        res = run_bass_kernel_spmd(nc, in_maps, core_ids=list(range(NCORES)),
                                   trace=trace)
    except ModuleNotFoundError:
        # NTFF profile hook unavailable in this environment.
        res = run_bass_kernel_spmd(nc, in_maps, core_ids=list(range(NCORES)),
                                   trace=False)
    out = np.concatenate([r["out"] for r in res.results], axis=0)
    return out, res


def kernel(X: np.ndarray) -> np.ndarray:
    X = np.asarray(X, dtype=np.float32)
    assert X.shape == (N, D)
    out, _ = _run(X, trace=False)
    # adj[i,i] = exp(-||x_i - x_i||/T) = exp(0) = 1 analytically; the device
    # path computes it through the same clamped-cancellation as any other
    # element, so pin the known-exact value.
    np.fill_diagonal(out, 1.0)
    return out
